# revision 11
# baseline (speedup 1.0000x reference)
"""Trainium2 Bass kernel for an encoder-decoder transformer block.

Sharding: data-parallel over batch (B=8 -> 8 cores, one batch element each).
Layout strategy (per core, S=2048 tokens, D=512):
  - activations live "transposed": [feature (partition chunks of 128), token];
    x -> h -> y share one buffer (each LN writes a slice only after the last
    read of that slice)
  - V is kept token-major [token, head*dv] and augmented with a ones column,
    so P@V also emits the softmax denominator (row 64 of the PSUM tile)
  - scores are computed transposed S^T[j(key), i(query)] so the exp output
    feeds the P@V matmul directly (contraction j on partitions); the two
    heads of a partition chunk run on distinct PE row groups concurrently
  - per-query normalization (1/denom * mask) applied to O^T via a gpsimd
    partition_broadcast of the scale row
  - layernorm stats via 1/512-vector matmuls (partition-axis sums), and
    rstd = exp(-0.5*ln(var+eps)) so ScalarE stays on one ACT table set
All matmul operands bf16 (f32 accumulation in PSUM); host packs/casts inputs.
"""

import numpy as np
import ml_dtypes

BF16 = ml_dtypes.bfloat16

B, S, D = 8, 2048, 512
H, DK, DV, DFF = 8, 64, 64, 2048
EPS = 1e-6
N_CORES = 8
P = 128
NDC = D // P          # 4 feature chunks
NSL = S // 512        # 4 token slices of 512
NSC = S // P          # 16 token chunks of 128
NFC = DFF // P        # 16 ffn chunks
VST = 66              # v_aug per-head stride (64 v + 1 ones + 1 pad)

# pcol column offsets (per-partition f32 params, [128, PCOLS])
C_BQ_S, C_BK_S, C_BO_S = 0, 4, 8
C_BQ_C, C_BK_C, C_BO_C = 12, 16, 20
C_B1, C_B2 = 24, 40
C_G1, C_BB1, C_G2, C_BB2, C_G3, C_BB3 = 44, 48, 52, 56, 60, 64
PCOLS = 68

_PROG = None  # (nc, input_names)


def _pack_dT(a):
    """[S, D] -> transposed chunked [128, NDC, S] (feature-major)."""
    t = np.ascontiguousarray(a.T)                    # [D, S]
    return np.ascontiguousarray(t.reshape(NDC, P, S).transpose(1, 0, 2))


def _pack_w(w):
    """[D, M] weight -> [128, D//128, M] (lhsT chunks on partitions)."""
    d, m = w.shape
    return np.ascontiguousarray(w.reshape(d // P, P, m).transpose(1, 0, 2))


def _pack_wo(wo):
    """[H*DV, D] -> [64, H, NDC, 128] for K=64 output-proj matmuls."""
    w = wo.reshape(H, DV, NDC, P)
    return np.ascontiguousarray(w.transpose(1, 0, 2, 3))


def _pack_col(v):
    """[n*128] -> [128, n] per-partition columns."""
    n = v.shape[0] // P
    return v.reshape(n, P).T


def _build_program():
    import concourse.bacc as bacc
    import concourse.tile as tile
    from concourse import mybir
    from contextlib import ExitStack

    f32 = mybir.dt.float32
    bf16 = mybir.dt.bfloat16
    EXPF = mybir.ActivationFunctionType.Exp
    LNF = mybir.ActivationFunctionType.Ln
    ADD = mybir.AluOpType.add
    SUBF = mybir.AluOpType.subtract
    MUL = mybir.AluOpType.mult
    MAX = mybir.AluOpType.max

    nc = bacc.Bacc("TRN2", target_bir_lowering=False, debug=False,
                   num_devices=N_CORES)

    def din(name, shape, dt):
        return nc.dram_tensor(name, shape, dt, kind="ExternalInput").ap()

    xT_d = din("xT", [P, NDC, S], bf16)
    enc_d = din("encT", [P, NDC, S], bf16)
    wq_s_d = din("wq_s", [P, NDC, 512], bf16)
    wk_s_d = din("wk_s", [P, NDC, 512], bf16)
    wv_s_d = din("wv_s", [P, NDC, 512], bf16)
    wo_s_d = din("wo_s", [DV, H, NDC, P], bf16)
    wq_c_d = din("wq_c", [P, NDC, 512], bf16)
    wk_c_d = din("wk_c", [P, NDC, 512], bf16)
    wv_c_d = din("wv_c", [P, NDC, 512], bf16)
    wo_c_d = din("wo_c", [DV, H, NDC, P], bf16)
    w1_d = din("w1", [P, NDC, DFF], bf16)
    w2_d = din("w2", [P, NFC, 512], bf16)
    pcol_d = din("pcol", [P, PCOLS], f32)
    mask_d = din("mask_r", [1, S], f32)
    bvs_d = din("bv_s_bc", [P, 512], f32)
    bvc_d = din("bv_c_bc", [P, 512], f32)
    out_d = nc.dram_tensor("outT", [P, NDC, S], f32, kind="ExternalOutput").ap()

    input_names = ["xT", "encT", "wq_s", "wk_s", "wv_s", "wo_s",
                   "wq_c", "wk_c", "wv_c", "wo_c", "w1", "w2", "pcol",
                   "mask_r", "bv_s_bc", "bv_c_bc"]

    with tile.TileContext(nc) as tc, ExitStack() as ctx:
        # ---------------- pools ----------------
        persist = ctx.enter_context(tc.tile_pool(name="persist", bufs=1))
        # PSUM: "mm" 2x1 bank + "sc" 3x2 banks = 8 banks
        psum_mm = ctx.enter_context(
            tc.tile_pool(name="psum_mm", bufs=2, space="PSUM"))
        psum_sc = ctx.enter_context(
            tc.tile_pool(name="psum_sc", bufs=3, space="PSUM"))
        rows = ctx.enter_context(tc.tile_pool(name="rows", bufs=2))
        recp = ctx.enter_context(tc.tile_pool(name="recp", bufs=4))
        bcab = ctx.enter_context(tc.tile_pool(name="bcab", bufs=2))
        bcnp = ctx.enter_context(tc.tile_pool(name="bcnp", bufs=3))
        tpool = ctx.enter_context(tc.tile_pool(name="tpool", bufs=6))
        wtp = ctx.enter_context(tc.tile_pool(name="wtp", bufs=2))
        expp = ctx.enter_context(tc.tile_pool(name="expp", bufs=4))
        stgp = ctx.enter_context(tc.tile_pool(name="stgp", bufs=3))
        encp = ctx.enter_context(tc.tile_pool(name="encp", bufs=6))
        outp = ctx.enter_context(tc.tile_pool(name="outp", bufs=5))

        # ---------------- persistent SBUF ----------------
        actT = persist.tile([P, NDC, S], bf16, tag="actT")  # x, then h, then y
        wq = persist.tile([P, NDC, 512], bf16, tag="wq")
        wk = persist.tile([P, NDC, 512], bf16, tag="wk")
        wv = persist.tile([P, NDC, 512], bf16, tag="wv")
        wo = persist.tile([DV, H, NDC, P], bf16, tag="wo")
        pcol = persist.tile([P, PCOLS], f32, tag="pcol")
        mask_r = persist.tile([1, S], f32, tag="mask_r")
        bv_s = persist.tile([P, 512], f32, tag="bv_s")
        bv_c = persist.tile([P, 512], f32, tag="bv_c")
        inv512 = persist.tile([P, 1], bf16, tag="inv512")

        nc.sync.dma_start(actT[:], xT_d[:])
        nc.sync.dma_start(wq[:], wq_s_d[:])
        nc.sync.dma_start(wk[:], wk_s_d[:])
        nc.sync.dma_start(wv[:], wv_s_d[:])
        nc.sync.dma_start(wo[:], wo_s_d[:])
        nc.sync.dma_start(pcol[:], pcol_d[:])
        nc.sync.dma_start(mask_r[:], mask_d[:])
        nc.sync.dma_start(bv_s[:], bvs_d[:])
        nc.sync.dma_start(bv_c[:], bvc_d[:])
        nc.vector.memset(inv512[:], 1.0 / 512.0)

        def pc(c):
            return pcol[:, c:c + 1]

        def isl_sl(t, dc, isl):
            return t[:, dc, isl * 512:(isl + 1) * 512]

        def rowstats_and_apply(stats, t_tiles, cg, cb, dest, dest_dt):
            """LayerNorm: stats psum [33,512] (row0=mean, row32=E[x^2]) ->
            per-token mu/rstd rows -> broadcast -> apply to the 4 chunks."""
            mu_bf = rows.tile([1, 512], bf16, tag="r_mu")
            nc.vector.tensor_copy(mu_bf[:], stats[0:1, :])
            m2 = rows.tile([1, 512], f32, tag="r_tmp", name="m2")
            nc.vector.tensor_mul(m2[:], mu_bf[:], mu_bf[:])
            var = rows.tile([1, 512], f32, tag="r_tmp", name="var")
            nc.vector.scalar_tensor_tensor(
                var[:], stats[32:33, :], EPS, m2[:], ADD, SUBF)
            lnv = rows.tile([1, 512], f32, tag="r_tmp", name="lnv")
            nc.scalar.activation(lnv[:], var[:], LNF)
            rstd = rows.tile([1, 512], bf16, tag="r_rstd")
            nc.scalar.activation(rstd[:], lnv[:], EXPF, scale=-0.5)
            bcA = bcab.tile([P, 512], bf16, tag="bcA")
            nc.gpsimd.partition_broadcast(bcA[:], rstd[:])
            bcB = bcab.tile([P, 512], bf16, tag="bcB")
            nc.gpsimd.partition_broadcast(bcB[:], mu_bf[:])
            for dm in range(NDC):
                w_ = wtp.tile([P, 512], bf16, tag="lnw")
                nc.vector.tensor_sub(w_[:], t_tiles[dm][:], bcB[:])
                z_ = wtp.tile([P, 512], dest_dt, tag="lnz")
                nc.vector.tensor_mul(z_[:], w_[:], bcA[:])
                nc.vector.tensor_scalar(
                    dest(dm), z_[:], pc(cg + dm), pc(cb + dm), MUL, ADD)

        def ln_stats_chunk(stats, t_, tsq_, dm):
            nc.tensor.matmul(stats[0:1, :], inv512[:], t_[:],
                             start=(dm == 0), stop=(dm == NDC - 1),
                             skip_group_check=True)
            nc.tensor.matmul(stats[32:33, :], inv512[:], tsq_[:],
                             start=(dm == 0), stop=(dm == NDC - 1),
                             skip_group_check=True)

        def qk_proj(w_sb, src, dst, cbias):
            """dst[:, mc, isl] = (W^T @ src) + bias for all mc/isl."""
            for mc in range(NDC):
                for isl in range(NSL):
                    ps = psum_sc.tile([P, 512], f32, tag="sc")
                    for dc in range(NDC):
                        nc.tensor.matmul(
                            ps[:], w_sb[:, dc, mc * P:(mc + 1) * P],
                            isl_sl(src, dc, isl),
                            start=(dc == 0), stop=(dc == NDC - 1))
                    nc.vector.tensor_scalar_add(
                        isl_sl(dst, mc, isl), ps[:], pc(cbias + mc))

        def v_write(ps, vaug, sc, bv):
            nc.vector.tensor_add(
                vaug[:, sc, :, 0:DV],
                ps[:].rearrange("p (h v) -> p h v", h=H),
                bv[:].rearrange("p (h v) -> p h v", h=H))

        def attention_isl(qT, kT, vaug, oT_of, use_mask, isl):
            for hp in range(NDC):
                pv = [psum_mm.tile([DV + 1, 512], f32, tag="mm",
                                   name=f"pv{k}")
                      for k in range(2)]  # head A, head B
                for jc in range(NSC):
                    sct = psum_sc.tile([P, 1024], f32, tag="sc")
                    for ab in range(2):  # adjacent A/B -> PE row groups
                        pb = ab * DV
                        nc.tensor.matmul(
                            sct[:, ab * 512:(ab + 1) * 512],
                            kT[pb:pb + DK, hp, jc * P:(jc + 1) * P],
                            qT[pb:pb + DK, hp, isl * 512:(isl + 1) * 512])
                    ex = expp.tile([P, 1024], bf16, tag="exp")
                    nc.scalar.activation(ex[:], sct[:], EXPF, scale=0.125)
                    for ab in range(2):
                        h = 2 * hp + ab
                        nc.tensor.matmul(
                            pv[ab][:], vaug[:, jc, h, 0:DV + 1],
                            ex[:, ab * 512:(ab + 1) * 512],
                            start=(jc == 0), stop=(jc == NSC - 1))
                for ab in range(2):
                    h = 2 * hp + ab
                    stg = stgp.tile([DV + 1, 512], bf16, tag="stg")
                    nc.vector.tensor_copy(stg[:], pv[ab][:])
                    rec = recp.tile([1, 512], f32, tag="rr", name="rec")
                    nc.vector.reciprocal(rec[:], stg[DV:DV + 1, :])
                    sca = recp.tile([1, 512], bf16, tag="rs", name="sca")
                    if use_mask:
                        nc.vector.tensor_mul(
                            sca[:], rec[:],
                            mask_r[:, isl * 512:(isl + 1) * 512])
                    else:
                        nc.vector.tensor_copy(sca[:], rec[:])
                    bcn = bcnp.tile([DV, 512], bf16, tag="bcn")
                    nc.gpsimd.partition_broadcast(bcn[:], sca[:])
                    nc.vector.tensor_mul(
                        oT_of(isl)[0:DV, h, :], stg[0:DV, :], bcn[:])

        def out_proj_ln(oT_of, cbo, res_src, cg, cb, dest_sb, isl_list):
            for isl in isl_list:
                t_tiles = []
                stats = psum_mm.tile([33, 512], f32, tag="mm")
                for dm in range(NDC):
                    ps = psum_sc.tile([P, 512], f32, tag="sc")
                    for h in range(H):
                        nc.tensor.matmul(
                            ps[:], wo[:, h, dm, :], oT_of(isl)[0:DV, h, :],
                            start=(h == 0), stop=(h == H - 1))
                    t_ = tpool.tile([P, 512], bf16, tag="t")
                    nc.vector.scalar_tensor_tensor(
                        t_[:], ps[:], pc(cbo + dm),
                        isl_sl(res_src, dm, isl), ADD, ADD)
                    t_tiles.append(t_)
                    tsq = wtp.tile([P, 512], bf16, tag="tsq")
                    nc.vector.tensor_mul(tsq[:], t_[:], t_[:])
                    ln_stats_chunk(stats, t_, tsq, dm)
                rowstats_and_apply(
                    stats, t_tiles, cg, cb,
                    lambda dm: isl_sl(dest_sb, dm, isl), bf16)

        # ================= phase 1: self-attention =======================
        with tc.tile_pool(name="qkv", bufs=1) as qkv, \
                tc.tile_pool(name="oTp", bufs=3) as oTp:
            qT = qkv.tile([P, NDC, S], bf16, tag="qT")
            kT = qkv.tile([P, NDC, S], bf16, tag="kT")
            vaug = qkv.tile([P, NSC, H, VST], bf16, tag="vaug")
            nc.vector.memset(vaug[:, :, :, DV:DV + 1], 1.0)

            qk_proj(wq, actT, qT, C_BQ_S)
            qk_proj(wk, actT, kT, C_BK_S)
            for sc in range(NSC):
                ps = psum_sc.tile([P, 512], f32, tag="sc")
                for dc in range(NDC):
                    nc.tensor.matmul(
                        ps[:], actT[:, dc, sc * P:(sc + 1) * P],
                        wv[:, dc, :],
                        start=(dc == 0), stop=(dc == NDC - 1))
                v_write(ps, vaug, sc, bv_s)

            oT_tiles = {}

            def oT_of(isl):
                if isl not in oT_tiles:
                    oT_tiles[isl] = oTp.tile([DV, H, 512], bf16, tag="oT",
                                             name=f"oT{isl}")
                return oT_tiles[isl]

            for isl in range(NSL):
                attention_isl(qT, kT, vaug, oT_of, True, isl)
                out_proj_ln(oT_of, C_BO_S, actT, C_G1, C_BB1, actT, [isl])
            oT_tiles.clear()

            # ============== phase 2: cross attention =====================
            nc.sync.dma_start(wq[:], wq_c_d[:])
            nc.sync.dma_start(wk[:], wk_c_d[:])
            nc.sync.dma_start(wv[:], wv_c_d[:])
            nc.sync.dma_start(wo[:], wo_c_d[:])

            qk_proj(wq, actT, qT, C_BQ_C)
            for isl in range(NSL):
                etiles = []
                for dc in range(NDC):
                    et = encp.tile([P, 512], bf16, tag="enc")
                    nc.sync.dma_start(et[:], enc_d[:, dc,
                                                   isl * 512:(isl + 1) * 512])
                    etiles.append(et)
                for mc in range(NDC):
                    ps = psum_sc.tile([P, 512], f32, tag="sc")
                    for dc in range(NDC):
                        nc.tensor.matmul(
                            ps[:], wk[:, dc, mc * P:(mc + 1) * P],
                            etiles[dc][:],
                            start=(dc == 0), stop=(dc == NDC - 1))
                    nc.vector.tensor_scalar_add(
                        isl_sl(kT, mc, isl), ps[:], pc(C_BK_C + mc))
                for sv in range(4):
                    sc = isl * 4 + sv
                    ps = psum_sc.tile([P, 512], f32, tag="sc")
                    for dc in range(NDC):
                        nc.tensor.matmul(
                            ps[:], etiles[dc][:, sv * P:(sv + 1) * P],
                            wv[:, dc, :],
                            start=(dc == 0), stop=(dc == NDC - 1))
                    v_write(ps, vaug, sc, bv_c)

            for isl in range(NSL):
                attention_isl(qT, kT, vaug, oT_of, False, isl)
                out_proj_ln(oT_of, C_BO_C, actT, C_G2, C_BB2, actT, [isl])
            oT_tiles.clear()

        # ================= phase 3: FFN + LN3 -> out =====================
        with tc.tile_pool(name="ffn", bufs=1) as ffn, \
                tc.tile_pool(name="f1p", bufs=2) as f1p:
            w1 = ffn.tile([P, NDC, DFF], bf16, tag="w1")
            w2 = ffn.tile([P, NFC, 512], bf16, tag="w2")
            nc.sync.dma_start(w1[:], w1_d[:])
            nc.sync.dma_start(w2[:], w2_d[:])

            for isl in range(NSL):
                f1 = f1p.tile([P, NFC, 512], bf16, tag="f1")
                for fm in range(NFC):
                    ps = psum_sc.tile([P, 512], f32, tag="sc")
                    for dc in range(NDC):
                        nc.tensor.matmul(
                            ps[:], w1[:, dc, fm * P:(fm + 1) * P],
                            isl_sl(actT, dc, isl),
                            start=(dc == 0), stop=(dc == NDC - 1))
                    nc.vector.tensor_scalar(
                        f1[:, fm, :], ps[:], pc(C_B1 + fm), 0.0, ADD, MAX)
                t_tiles = []
                stats = psum_mm.tile([33, 512], f32, tag="mm")
                for dm in range(NDC):
                    ps = psum_mm.tile([P, 512], f32, tag="mm")
                    for fc in range(NFC):
                        nc.tensor.matmul(
                            ps[:], w2[:, fc, dm * P:(dm + 1) * P],
                            f1[:, fc, :],
                            start=(fc == 0), stop=(fc == NFC - 1))
                    t_ = tpool.tile([P, 512], bf16, tag="t")
                    nc.vector.scalar_tensor_tensor(
                        t_[:], ps[:], pc(C_B2 + dm),
                        isl_sl(actT, dm, isl), ADD, ADD)
                    t_tiles.append(t_)
                    tsq = wtp.tile([P, 512], bf16, tag="tsq")
                    nc.vector.tensor_mul(tsq[:], t_[:], t_[:])
                    ln_stats_chunk(stats, t_, tsq, dm)
                otiles = [outp.tile([P, 512], f32, tag="out", name=f"ot{k}")
                          for k in range(NDC)]
                rowstats_and_apply(stats, t_tiles, C_G3, C_BB3,
                                   lambda dm: otiles[dm][:], f32)
                for dm in range(NDC):
                    nc.sync.dma_start(
                        out_d[:, dm, isl * 512:(isl + 1) * 512], otiles[dm][:])

    # Pin Exp and Ln to the single shared ACT table set so the scalar
    # engine never reloads tables mid-kernel (exp<->ln thrash costs ~2.7us
    # per reload and stalls the PE pipeline behind it).
    import concourse.bacc as bacc_mod
    orig_tables = bacc_mod.get_activation_tables

    def pinned_tables(arch):
        t = dict(orig_tables(arch))
        for name in ("exp_and_others", "exp_and_friends", "natural_log"):
            if name in t:
                t[name] = set()
        return t

    bacc_mod.get_activation_tables = pinned_tables
    try:
        nc.compile()
    finally:
        bacc_mod.get_activation_tables = orig_tables
    return nc, input_names


def _get_program():
    global _PROG
    if _PROG is None:
        _PROG = _build_program()
    return _PROG


def _pack_inputs(inputs):
    f = {k: np.asarray(v, dtype=np.float32) for k, v in inputs.items()}

    def hcat(w):  # [H, D, dk] -> [D, H*dk]
        return np.ascontiguousarray(w.transpose(1, 0, 2).reshape(D, -1))

    pcol = np.zeros((P, PCOLS), np.float32)
    pcol[:, C_BQ_S:C_BQ_S + 4] = _pack_col(f["bq_s"].reshape(-1))
    pcol[:, C_BK_S:C_BK_S + 4] = _pack_col(f["bk_s"].reshape(-1))
    pcol[:, C_BO_S:C_BO_S + 4] = _pack_col(f["bo_s"])
    pcol[:, C_BQ_C:C_BQ_C + 4] = _pack_col(f["bq_c"].reshape(-1))
    pcol[:, C_BK_C:C_BK_C + 4] = _pack_col(f["bk_c"].reshape(-1))
    pcol[:, C_BO_C:C_BO_C + 4] = _pack_col(f["bo_c"])
    pcol[:, C_B1:C_B1 + 16] = _pack_col(f["b1"])
    pcol[:, C_B2:C_B2 + 4] = _pack_col(f["b2"])
    pcol[:, C_G1:C_G1 + 4] = _pack_col(f["ln1_g"])
    pcol[:, C_BB1:C_BB1 + 4] = _pack_col(f["ln1_b"])
    pcol[:, C_G2:C_G2 + 4] = _pack_col(f["ln2_g"])
    pcol[:, C_BB2:C_BB2 + 4] = _pack_col(f["ln2_b"])
    pcol[:, C_G3:C_G3 + 4] = _pack_col(f["ln3_g"])
    pcol[:, C_BB3:C_BB3 + 4] = _pack_col(f["ln3_b"])

    mask_r = np.ascontiguousarray(f["mask"].reshape(1, S))
    bvs = np.broadcast_to(f["bv_s"].reshape(1, -1), (P, H * DV)).copy()
    bvc = np.broadcast_to(f["bv_c"].reshape(1, -1), (P, H * DV)).copy()

    shared = {
        "wq_s": _pack_w(hcat(f["Wq_s"])).astype(BF16),
        "wk_s": _pack_w(hcat(f["Wk_s"])).astype(BF16),
        "wv_s": _pack_w(hcat(f["Wv_s"])).astype(BF16),
        "wo_s": _pack_wo(f["Wo_s"]).astype(BF16),
        "wq_c": _pack_w(hcat(f["Wq_c"])).astype(BF16),
        "wk_c": _pack_w(hcat(f["Wk_c"])).astype(BF16),
        "wv_c": _pack_w(hcat(f["Wv_c"])).astype(BF16),
        "wo_c": _pack_wo(f["Wo_c"]).astype(BF16),
        "w1": _pack_w(f["W1"]).astype(BF16),
        "w2": _pack_w(f["W2"]).astype(BF16),
        "pcol": pcol, "mask_r": mask_r,
        "bv_s_bc": bvs, "bv_c_bc": bvc,
    }
    in_maps = []
    for b in range(N_CORES):
        m = dict(shared)
        m["xT"] = _pack_dT(f["x"][b]).astype(BF16)
        m["encT"] = _pack_dT(f["encoding"][b]).astype(BF16)
        in_maps.append(m)
    return in_maps


def kernel(**inputs):
    from concourse.bass_utils import run_bass_kernel_spmd

    nc, _ = _get_program()
    in_maps = _pack_inputs(inputs)
    res = run_bass_kernel_spmd(nc, in_maps, list(range(N_CORES)))
    out = np.empty((B, S, D), np.float32)
    for b in range(N_CORES):
        oT = np.asarray(res.results[b]["outT"], dtype=np.float32)
        out[b] = oT.reshape(P, NDC, S).transpose(1, 0, 2).reshape(D, S).T
    return out
